# revision 6
# baseline (speedup 1.0000x reference)
"""Trainium2 Bass kernel v3 for the Clements mesh chain (N=512).

Same two-phase architecture as v2 (banded chunk transposes + panel sweep)
with three structural speedups:

  Phase A (banded chain, per-core):
    State is held in 3-plane complex form [imneg | re | im] per (E/O, b)
    so each chain op covers the (re,im) superplane at once with a single
    per-partition scalar: a*x -> re|im = ar (.) [re|im] + ai (.) [imneg|re].
    Per layer: 4 Act starters + 12 DVE STT + 4 DVE negates (vs 8 Act +
    24 DVE in v2).  E0/E1 (and O0/O1) live in one tile so the PE shift
    needs 4 weight loads per H layer instead of 6.

  Phase B (panel sweep): all tensors in fp16 (PE streams 1 cyc/row vs
    fp32r's 2-pass; DMA + AllGather bytes halve).  Bounce-expansion DMAs
    are merged (2 writes + 2 dense reads per chunk) to cut DMA-issue
    sequencer time.

  Collectives stay on GpSimd (NRT straight-line requirement) but GpSimd
  otherwise only carries phase-B negate work that depends on the gather
  anyway; per-layer memsets are gone (guard zeros persist; layer-0 chains
  write the full band range to scrub stale tile data).
"""

import numpy as np

N = 512
S = 256
NCORES = 8
NCH = 32             # chunks
SCH = S // NCH       # 8 steps per chunk
CPC = NCH // NCORES  # 4 chunks per core
LAY = 2 * SCH        # 16 layers per chunk (H.T, G.T per reversed step)
NLAY = CPC * LAY     # 64 layers per core
PB = 2
NSLOT = 8            # coefficient scalars per (layer, b)
BW = 40              # band slots per plane
PADB = 19            # slot of diagonal for E rows (O rows: PADB+1)
LO, HI = 2, BW - 2   # chain op range; guard slots outside stay zero
RS = 416             # skew bounce row stride
WIN = 384            # dense window per K-block pair
PCOLS = 128          # panel columns per core (cores c, c+4 duplicate)
WSTART = [0, 96, 192, 288]   # uniform stride so dense reads merge

MERGED_SHIFTS = True     # 4 weight loads per H layer (subrange psum accum)
GPSIMD_IMNEG = True      # phase-B negates on Pool (fallback: DVE)
DEBUG_LEAVES = False     # add a dbgleaf output with the phase-A leaf bands
DEBUG_PHASEB = False     # dump dense tiles and panel state for chunk 0


# ----------------------------------------------------------------------------
# Host math
# ----------------------------------------------------------------------------

def _mmi_2x2(loss, imb):
    a = np.sqrt(1.0 - loss.astype(np.float64))
    t = a * np.sqrt(0.5 + imb.astype(np.float64))
    r = a * np.sqrt(0.5 - imb.astype(np.float64))
    m = np.zeros(loss.shape + (2, 2), np.complex128)
    m[..., 0, 0] = t
    m[..., 1, 1] = t
    m[..., 0, 1] = 1j * r
    m[..., 1, 0] = 1j * r
    return m


def _pc_vec(theta, loss):
    return np.sqrt(1.0 - loss.astype(np.float64)) * np.exp(1j * theta.astype(np.float64))


def host_fold_layers(inputs):
    th = np.asarray(inputs["thetas_full"], np.float64)
    lp = np.asarray(inputs["pc_losses_full"], np.float64)
    tio = np.asarray(inputs["thetas_inout"], np.float64)
    lio = np.asarray(inputs["pc_losses_inout"], np.float64)
    le = np.asarray(inputs["mmi_losses_even"], np.float64)
    ie = np.asarray(inputs["mmi_imb_even"], np.float64)
    lo = np.asarray(inputs["mmi_losses_odd"], np.float64)
    io = np.asarray(inputs["mmi_imb_odd"], np.float64)

    G = np.zeros((S, 256, 2, 2), np.complex128)
    Hp = np.zeros((S, 255, 2, 2), np.complex128)
    h_edge = np.zeros((S, 2), np.complex128)

    for s in range(S):
        e1 = _mmi_2x2(le[2 * s], ie[2 * s])
        e2 = _mmi_2x2(le[2 * s + 1], ie[2 * s + 1])
        a0 = _pc_vec(th[2 * s], lp[2 * s]).reshape(256, 2)
        G[s] = e2 @ (a0[:, :, None] * e1)

        o1 = _mmi_2x2(lo[2 * s], io[2 * s])
        o2 = _mmi_2x2(lo[2 * s + 1], io[2 * s + 1])
        a1 = _pc_vec(th[2 * s + 1], lp[2 * s + 1])
        a1p = a1[1:-1].reshape(255, 2)
        Hp[s] = o2 @ (a1p[:, :, None] * o1)
        h_edge[s, 0] = a1[0]
        h_edge[s, 1] = a1[-1]

    ain = _pc_vec(tio[0], lio[0]).reshape(256, 2)
    G[0] = G[0] * ain[:, None, :]
    aout = _pc_vec(tio[1], lio[1])
    Hp[S - 1] = aout[1:-1].reshape(255, 2)[:, :, None] * Hp[S - 1]
    h_edge[S - 1, 0] *= aout[0]
    h_edge[S - 1, 1] *= aout[-1]
    return G, Hp, h_edge


def _h_coeffs(Hp_s, edge_s, transpose):
    """One H layer -> (c00, c01, d10, d11) arrays [256], dst-partition aligned."""
    h00, h01 = Hp_s[:, 0, 0], Hp_s[:, 0, 1]
    h10, h11 = Hp_s[:, 1, 0], Hp_s[:, 1, 1]
    if transpose:
        h01, h10 = h10, h01
    c00 = np.zeros(256, np.complex128)
    c01 = np.zeros(256, np.complex128)
    d10 = np.zeros(256, np.complex128)
    d11 = np.zeros(256, np.complex128)
    c00[:255] = h00
    c00[255] = edge_s[1]
    c01[:255] = h01
    d11[1:] = h11
    d11[0] = edge_s[0]
    d10[1:] = h10
    return c00, c01, d10, d11


def _slots8(a, b, c, d):
    """Four complex [256] coefficient vectors -> [256, 8] f32 (r,i pairs)."""
    return np.stack([a.real, a.imag, b.real, b.imag,
                     c.real, c.imag, d.real, d.imag], axis=-1).astype(np.float32)


def chunk_scales(inputs):
    """Per-chunk power-of-2 scale exponents k_j (and their total).

    The mesh is lossy (~0.9875 amplitude per primitive layer), so the fp16
    panel would underflow after ~20 chunks.  Scale chunk j's factor by
    2**k_j, with k_j chosen so the cumulative scale tracks the cumulative
    gain estimate; the host divides the output by 2**sum(k_j) (exact).
    """
    lp = np.asarray(inputs["pc_losses_full"], np.float64)
    lio = np.asarray(inputs["pc_losses_inout"], np.float64)
    le = np.asarray(inputs["mmi_losses_even"], np.float64)
    lo = np.asarray(inputs["mmi_losses_odd"], np.float64)
    lg = np.zeros(NCH)
    for j in range(NCH):
        acc = 0.0
        for s in range(j * SCH, (j + 1) * SCH):
            for v in (le[2 * s], le[2 * s + 1], lp[2 * s], lp[2 * s + 1],
                      lo[2 * s], lo[2 * s + 1]):
                acc += 0.5 * np.mean(np.log2(1.0 - v))
        lg[j] = acc
    lg[0] += 0.5 * np.mean(np.log2(1.0 - lio[0]))
    lg[NCH - 1] += 0.5 * np.mean(np.log2(1.0 - lio[1]))
    cum = np.cumsum(lg)
    kcum = np.round(-cum).astype(int)
    k = np.diff(np.concatenate([[0], kcum]))
    return k, kcum[-1]


def host_coeff_core(inputs, core, folded=None):
    """Per-core coefficient array [128, NLAY*PB*NSLOT] for superplane chains.

    H layer slots: (c00, c01) for nO; (d11, d10) for nE.
    G layer slots: (gT00, gT01) for nE; (gT10, gT11) for nO.
    """
    G, Hp, h_edge = folded if folded is not None else host_fold_layers(inputs)
    kvec, _ = chunk_scales(inputs)
    layers = []
    for ch in range(CPC):
        j = 8 * ch + core
        scale = np.float32(2.0 ** kvec[j])
        s0, s1 = j * SCH, (j + 1) * SCH
        for s in range(s1 - 1, s0 - 1, -1):
            c00, c01, d10, d11 = _h_coeffs(Hp[s], h_edge[s], transpose=True)
            sc = scale if s == s1 - 1 else np.float32(1.0)
            layers.append(_slots8(c01, c00, d10, d11) * sc)
            gT = G[s].transpose(0, 2, 1)
            layers.append(_slots8(gT[:, 0, 0], gT[:, 0, 1], gT[:, 1, 0], gT[:, 1, 1]))
    arr = np.stack(layers, axis=0)                    # [NLAY, 256, 8]
    arr = arr.reshape(NLAY, PB, 128, NSLOT).transpose(2, 0, 1, 3)
    return np.ascontiguousarray(arr.reshape(128, NLAY * PB * NSLOT))


def host_shift_mats():
    """Constant PE shift matrices [128, 4*128] f32 (UP, SELA, DN, SELB)."""
    m = np.zeros((4, 128, 128), np.float32)
    for i in range(127):
        m[0, i + 1, i] = 1.0      # UP: out[m] = in[m+1]
        m[2, i, i + 1] = 1.0      # DN: out[m] = in[m-1]
    m[1, 0, 127] = 1.0            # SELA: out[127] = in[0]
    m[3, 127, 0] = 1.0            # SELB: out[0] = in[127]
    return np.ascontiguousarray(m.transpose(1, 0, 2).reshape(128, 512))


def host_panel_init(core):
    """Identity panel [4, 128, 3*PCOLS] fp16 (imneg|re|im), natural rows."""
    arr = np.zeros((4, 128, 3 * PCOLS), np.float16)
    colbase = (core % 4) * PCOLS
    for j in range(PCOLS):
        row = colbase + j
        b, p = divmod(row, 128)
        arr[b, p, PCOLS + j] = 1.0   # re plane
    return arr


# ----------------------------------------------------------------------------
# Device program
# ----------------------------------------------------------------------------

def build_program_v3():
    import concourse.bass as bass
    import concourse.tile as tile
    from concourse import bacc, mybir

    f32 = mybir.dt.float32
    f16 = mybir.dt.float16
    MUL = mybir.AluOpType.mult
    ADD = mybir.AluOpType.add

    nc = bacc.Bacc("TRN2", target_bir_lowering=False, debug=False,
                   num_devices=NCORES)
    coef_d = nc.dram_tensor("coef", [128, NLAY * PB * NSLOT], f32,
                            kind="ExternalInput")
    pinit_d = nc.dram_tensor("pinit", [4, 128, 3 * PCOLS], f16,
                             kind="ExternalInput")
    shm_d = nc.dram_tensor("shmats", [128, 4 * 128], f32, kind="ExternalInput")
    NBOUNCE = 12
    bounce_d = nc.dram_tensor("bouncez", [NBOUNCE, 2 * 512 * RS], f16,
                              kind="ExternalInput")
    out_d = nc.dram_tensor("mout", [4, 128, 2 * PCOLS], f16,
                           kind="ExternalOutput")
    dbg_d = None
    if DEBUG_LEAVES:
        dbg_d = nc.dram_tensor("dbgleaf", [CPC, 128, 4 * 2 * BW], f16,
                               kind="ExternalOutput")
    dbgB_d = None
    if DEBUG_PHASEB:
        dbgB_d = nc.dram_tensor("dbgB", [2 + 4, 128, 2 * 2 * WIN], f16,
                                kind="ExternalOutput")

    W3 = 3 * BW   # planes (imneg, re, im) per (name, b)

    with tile.TileContext(nc) as tc:
        with (
            tc.tile_pool(name="coefp", bufs=1) as coefp,
            tc.tile_pool(name="leafp", bufs=2) as leafp,
            tc.tile_pool(name="rndp", bufs=2) as rndp,
            tc.tile_pool(name="densep", bufs=3) as densep,
            tc.tile_pool(name="panelp", bufs=2) as panelp,
            tc.tile_pool(name="psumA", bufs=1, space="PSUM") as psumA,
            tc.tile_pool(name="psumB", bufs=1, space="PSUM") as psumB,
            tc.tile_pool(name="dramp", bufs=1, space="DRAM") as dramp,
            tc.tile_pool(name="bouncep", bufs=1, space="DRAM") as bouncep,
        ):
            coef = coefp.tile([128, NLAY * PB * NSLOT], f32)
            nc.sync.dma_start(coef[:], coef_d.ap())
            shm = coefp.tile([128, 4 * 128], f32)
            nc.sync.dma_start(shm[:], shm_d.ap())
            SH_UP, SH_SELA, SH_DN, SH_SELB = (
                shm[:, 128 * i:128 * (i + 1)] for i in range(4))

            panel = {}
            for b in range(4):
                t = panelp.tile([128, 3 * PCOLS], f16, tag=f"P{b}",
                                name=f"panel_init_{b}")
                nc.sync.dma_start(t[:], pinit_d.ap()[b])
                panel[b] = t

            def csc(lay, b, slot):
                idx = (lay * PB + b) * NSLOT + slot
                return coef[:, idx:idx + 1]

            # --- phase A: padded 1-dim plane layout -------------------
            # state tile: [pad2 | b0:(imneg,re,im) | b1:(imneg,re,im) | pad2]
            # All chain ops are contiguous 1-dim superplane slices covering
            # full planes; boundary bleed lands on slots that are always
            # truly zero (band halfwidth <= 17 < guard offsets +-19).
            PAD = 2
            TW = 2 * W3 + 2 * PAD

            def SPo(t, b, off=0):       # [re|im] superplane of half b
                base = PAD + b * W3 + BW + off
                return t[:, base:base + 2 * BW]

            def SPn(t, b, off=0):       # [imneg|re] superplane
                base = PAD + b * W3 + off
                return t[:, base:base + 2 * BW]

            def PLi(t, b):              # imneg plane
                base = PAD + b * W3
                return t[:, base:base + BW]

            def PLm(t, b):              # im plane
                base = PAD + b * W3 + 2 * BW
                return t[:, base:base + BW]

            shE = psumA.tile([128, TW], f32, tag="shE", name="shE")
            shO = psumA.tile([128, TW], f32, tag="shO", name="shO")
            nc.vector.memset(shE[:, 0:PAD], 0.0)
            nc.vector.memset(shE[:, TW - PAD:TW], 0.0)
            nc.vector.memset(shO[:, 0:PAD], 0.0)
            nc.vector.memset(shO[:, TW - PAD:TW], 0.0)

            def csc(lay, b, slot):
                idx = (lay * PB + b) * NSLOT + slot
                return coef[:, idx:idx + 1]

            def chain(lay, b, dst, s0, s1, base):
                """dst_SP = c0*s0 + c1*s1 (complex, superplane form)."""
                d = SPo(dst, b)
                nc.scalar.mul(d, s0[0], csc(lay, b, base + 0))
                nc.vector.scalar_tensor_tensor(
                    d, s0[1], csc(lay, b, base + 1), d, MUL, ADD)
                nc.vector.scalar_tensor_tensor(
                    d, s1[0], csc(lay, b, base + 2), d, MUL, ADD)
                nc.vector.scalar_tensor_tensor(
                    d, s1[1], csc(lay, b, base + 3), d, MUL, ADD)

            def neg_both(t):
                # imneg planes of both halves in one 2-dim op
                v = t[:, PAD:PAD + 2 * W3].rearrange("p (n w) -> p n w", n=2)
                nc.scalar.mul(v[:, :, 0:BW], v[:, :, 2 * BW:3 * BW], -1.0)

            # bounce buffers (zeroed once; band regions overwritten per use)
            PL = 512 * RS
            bounces = [bounce_d.ap()[bi] for bi in range(NBOUNCE)]

            gathered = []
            panel_box = {"panel": panel}

            def emit_B_writes(j):
                rnd_i, owner = j // 8, j % 8
                gat = gathered[rnd_i]
                bt = bounces[j % NBOUNCE].tensor
                bof = bounces[j % NBOUNCE].offset
                gv = gat[owner].rearrange("n p (r w) -> n p r w", r=2)
                nc.sync.dma_start(
                    bass.AP(bt, bof, [[256 * RS, 2], [2 * RS, 128],
                                      [PL, 2], [1, BW]]),
                    gv[0:2])
                nc.sync.dma_start(
                    bass.AP(bt, bof + RS - 1, [[256 * RS, 2], [2 * RS, 128],
                                               [PL, 2], [1, BW]]),
                    gv[2:4])

            def emit_B_dmas(j):
                bt = bounces[j % NBOUNCE].tensor
                bof = bounces[j % NBOUNCE].offset
                dt_ = densep.tile([128, 4 * 2 * WIN], f16, tag="D",
                                  name=f"dense{j}")
                dv = dt_[:].rearrange("p (r pl w) -> p r pl w", r=4, pl=2)
                for pl in range(2):
                    nc.sync.dma_start(
                        dv[:, :, pl],
                        bass.AP(bt, bof + WSTART[0] + PADB + pl * PL,
                                [[RS - 1, 128],
                                 [128 * (RS - 1) + 96, 4], [1, WIN]]))
                return dt_

            def emit_B_mms(j, dense):
                panel = panel_box["panel"]

                def lhs(b_, plane, moff):
                    base = (b_ * 2 + plane) * WIN + moff
                    return dense[:, base:base + 128]

                newp = {}
                tailj = j >= NCH - 8
                for m in range(4):
                    pool = psumB
                    ps = pool.tile([128, 2 * PCOLS], f32, tag=f"ps{m}",
                                   name=f"psum{j}_{m}")
                    mms = []
                    for pl in range(2):
                        for b_ in (m, m - 1, m + 1):
                            if not 0 <= b_ < 4:
                                continue
                            moff = 128 * m - WSTART[b_]
                            if pl == 0:
                                mms.append((lhs(b_, 0, moff),
                                            panel[b_][:, PCOLS:3 * PCOLS]))
                            else:
                                mms.append((lhs(b_, 1, moff),
                                            panel[b_][:, 0:2 * PCOLS]))
                    for i_, (lh, rh) in enumerate(mms):
                        nc.tensor.matmul(ps[:], lh, rh,
                                         start=(i_ == 0),
                                         stop=(i_ == len(mms) - 1))
                    # copy back this m-block immediately so later m groups
                    # overlap the psum->panel traffic (Act), and negate from
                    # psum directly (no DVE hop on the chunk critical path)
                    np_ = panelp.tile([128, 3 * PCOLS], f16, tag=f"P{m}",
                                      name=f"panel{j}_{m}")
                    if tailj and m % 2 == 1:
                        nc.vector.tensor_copy(np_[:, PCOLS:3 * PCOLS], ps[:])
                        nc.scalar.mul(np_[:, 0:PCOLS],
                                      ps[:, PCOLS:2 * PCOLS], -1.0)
                    else:
                        nc.scalar.copy(np_[:, PCOLS:3 * PCOLS], ps[:])
                        nc.vector.tensor_scalar_mul(
                            np_[:, 0:PCOLS], ps[:, PCOLS:2 * PCOLS], -1.0)
                    newp[m] = np_
                panel_box["panel"] = newp

            lay = 0
            for r in range(CPC):
                ch = r
                rbase = r * SCH * 2
                with tc.tile_wait_until(rbase - 0.2):
                    TE = leafp.tile([128, TW], f32, tag="TE", name=f"c{ch}_E")
                    TO = leafp.tile([128, TW], f32, tag="TO", name=f"c{ch}_O")
                    nc.vector.memset(TE[:], 0.0)
                    nc.vector.memset(TO[:], 0.0)
                    for b in range(2):
                        p0 = PAD + b * W3 + BW + PADB
                        nc.vector.memset(TE[:, p0:p0 + 1], 1.0)
                        nc.vector.memset(TO[:, p0 + 1:p0 + 2], 1.0)

                for t_step in range(SCH):
                    SL = rbase + 2 * t_step
                    # ---- H-type layer ----
                    ctxH = tc.tile_wait_until(SL)
                    ctxH.__enter__()
                    nc.tensor.matmul(shE[:, PAD:PAD + 2 * W3], SH_UP,
                                     TE[:, PAD:PAD + 2 * W3], start=True,
                                     stop=False, skip_group_check=True)
                    nc.tensor.matmul(shE[64:128, PAD:PAD + W3],
                                     SH_SELA[:, 64:128],
                                     TE[:, PAD + W3:PAD + 2 * W3],
                                     start=False, stop=True,
                                     skip_group_check=True)
                    nc.tensor.matmul(shO[:, PAD:PAD + 2 * W3], SH_DN,
                                     TO[:, PAD:PAD + 2 * W3], start=True,
                                     stop=False, skip_group_check=True)
                    nc.tensor.matmul(shO[0:32, PAD + W3:PAD + 2 * W3],
                                     SH_SELB[:, 0:32],
                                     TO[:, PAD:PAD + W3],
                                     start=False, stop=True,
                                     skip_group_check=True)

                    nTE = leafp.tile([128, TW], f32, tag="TE",
                                     name=f"h{ch}_{t_step}_E")
                    nTO = leafp.tile([128, TW], f32, tag="TO",
                                     name=f"h{ch}_{t_step}_O")
                    if ch == 0 and t_step == 0:
                        nc.vector.memset(nTE[:], 0.0)
                        nc.vector.memset(nTO[:], 0.0)
                    for b in range(2):
                        chain(lay, b, nTO, (SPo(shE, b, -2), SPn(shE, b, -2)),
                              (SPo(TO, b), SPn(TO, b)), 0)
                        chain(lay, b, nTE, (SPo(shO, b, 2), SPn(shO, b, 2)),
                              (SPo(TE, b), SPn(TE, b)), 4)
                    neg_both(nTO)
                    neg_both(nTE)
                    TE, TO = nTE, nTO
                    lay += 1
                    ctxH.__exit__(None, None, None)

                    # ---- G-type layer ----
                    ctxG = tc.tile_wait_until(SL + 1)
                    ctxG.__enter__()
                    nTE = leafp.tile([128, TW], f32, tag="TE",
                                     name=f"g{ch}_{t_step}_E")
                    nTO = leafp.tile([128, TW], f32, tag="TO",
                                     name=f"g{ch}_{t_step}_O")
                    for b in range(2):
                        chain(lay, b, nTE, (SPo(TE, b), SPn(TE, b)),
                              (SPo(TO, b), SPn(TO, b)), 0)
                        chain(lay, b, nTO, (SPo(TE, b), SPn(TE, b)),
                              (SPo(TO, b), SPn(TO, b)), 4)
                    neg_both(nTE)
                    neg_both(nTO)
                    TE, TO = nTE, nTO
                    lay += 1
                    ctxG.__exit__(None, None, None)

                # leaf -> fp16 (re|im planes only), order (E0, E1, O0, O1)
                ctxS = tc.tile_wait_until(rbase + 2 * SCH - 0.8)
                ctxS.__enter__()
                rnd = rndp.tile([128, 4 * 2 * BW], f16, tag="rnd",
                                name=f"rnd{ch}")
                for b in range(2):
                    nc.vector.tensor_copy(
                        rnd[:, b * 2 * BW:(b + 1) * 2 * BW], SPo(TE, b))
                    nc.vector.tensor_copy(
                        rnd[:, (2 + b) * 2 * BW:(3 + b) * 2 * BW], SPo(TO, b))
                send = dramp.tile([4, 128, 2 * BW], f16, name=f"send{ch}")
                nc.scalar.dma_start(
                    send[:].rearrange("n p w -> p n w"),
                    rnd[:].rearrange("p (n w) -> p n w", n=4))
                if DEBUG_LEAVES:
                    nc.scalar.dma_start(dbg_d.ap()[ch], rnd[:])
                gat = dramp.tile([NCORES, 4, 128, 2 * BW], f16,
                                 name=f"gat{ch}", addr_space="Shared")
                nc.gpsimd.collective_compute(
                    "AllGather", mybir.AluOpType.bypass,
                    replica_groups=[list(range(NCORES))],
                    ins=[send.opt()], outs=[gat.opt()],
                )
                gathered.append(gat)
                ctxS.__exit__(None, None, None)

            # drain round 3 (and anything left)
            # phase B on an absolute-slot schedule: round rj's chunks run
            # during round rj+1, starting ~2 layers in (gather surely done);
            # each chunk's write+read group issues 2.5 layers before its mms
            for j in range(NCH):
                rj, k = j // 8, j % 8
                base = 16 * (rj + 1)
                with tc.tile_wait_until(base + 4 + 2 * k):
                    emit_B_writes(j)
                    dense_j = emit_B_dmas(j)
                with tc.tile_wait_until(base + 10 + 2 * k):
                    emit_B_mms(j, dense_j)

            panel = panel_box["panel"]
            for b in range(4):
                nc.sync.dma_start(out_d.ap()[b], panel[b][:, PCOLS:3 * PCOLS])

    nc.compile()
    return nc


# ----------------------------------------------------------------------------
# Entry point
# ----------------------------------------------------------------------------

def assemble_output(per_core):
    """per_core: list (cores 0..3 used) of fp16 [4, 128, 2*PCOLS] -> [N,N] c64."""
    M = np.zeros((N, N), np.complex64)
    for c in range(4):
        arr = np.asarray(per_core[c], np.float32)
        cols = slice(c * PCOLS, (c + 1) * PCOLS)
        for b in range(4):
            rows = slice(b * 128, (b + 1) * 128)
            M[rows, cols] = arr[b, :, 0:PCOLS] + 1j * arr[b, :, PCOLS:2 * PCOLS]
    return M


_CACHE = {}


def kernel(**inputs) -> np.ndarray:
    import os

    from concourse.bass_utils import run_bass_kernel_spmd

    folded = host_fold_layers(inputs)
    if "nc" not in _CACHE:
        _CACHE["nc"] = build_program_v3()
    nc = _CACHE["nc"]

    shm = host_shift_mats()
    if "bz" not in _CACHE:
        _CACHE["bz"] = np.zeros((12, 2 * 512 * RS), np.float16)
    bz = _CACHE["bz"]
    in_maps = [
        {"coef": host_coeff_core(inputs, c, folded),
         "pinit": host_panel_init(c), "shmats": shm, "bouncez": bz}
        for c in range(NCORES)
    ]
    trace = bool(os.environ.get("KERNEL_TRACE"))
    res = run_bass_kernel_spmd(nc, in_maps, core_ids=list(range(NCORES)),
                               trace=trace)
    if res.exec_time_ns is not None:
        print(f"HW exec time: {res.exec_time_ns} ns")
    _, ktot = chunk_scales(inputs)
    out = assemble_output([r["mout"] for r in res.results])
    return (out * np.float32(2.0 ** (-ktot))).astype(np.complex64)
